# revision 29
# baseline (speedup 1.0000x reference)
"""Trainium2 Bass kernel for bidirectional masked-LSTM + attention pooling + FC head.

Problem (hardcoded shapes): B=64, T=512, E=256, H=512, OH=1024.
  - x [B,T,E] f32, lengths [B] i32, attn_w [T] f32
  - per-direction LSTM weights Wih [4H,E], Whh [4H,H], biases [4H]
  - fc1 [OH,2H]+[OH], fc2 [T,OH]+[T]
  - out: logits [B,T] f32, padded positions = -1e30

Sharding: 8 cores = 4 batch groups (16 seqs) x 2 directions. Each core runs one
direction's full 512-step recurrence for its 16 sequences. Attention pooling is
folded into the recurrence as a masked weighted accumulate (per-(t,b) scale
table precomputed on host, which also implements sequence reversal masking for
the backward direction). The FC head runs on every core; forward/backward
pooled partials are combined with a pairwise AllReduce.

Layouts (per core):
  h "hidden-tiled" [128, K_CH*16]: h[b, hid] at partition hid%128, col (hid//128)*16+b.
  gates PSUM tiled [128, m*16+b] per gate-chunk m (gate g=m*128+p), gate order
  permuted to [i, f, o, g] so i,f share one sigmoid and g is one tanh.

Schedule: the input projection xp = x@Wih.T + bias is streamed INSIDE the
recurrence: while block nb executes, block nb+1's projection matmuls run in
the PE's idle tail slots and the PSUM->SBUF evacuation (bias add, f16 cast)
runs on the idle Vector engine, writing straight into the inactive xpb SBUF
buffer (no DRAM round trip). Per step, the xp addend is folded into PSUM via
an identity matmul (runs during the previous step's elementwise tail), then
gate groups run i,f -> g -> o so sigmoid(i,f), f*c, tanh(g), i*g and the
c-update overlap the remaining matmul groups; only sigmoid(o) -> tanh(c) -> h
stays exposed. Each PSUM tile is padded to a full 2KB bank so the per-parity
tiles live in distinct banks (matmul start=True clears a whole bank).
"""

import os

import numpy as np

import concourse.bass as bass
import concourse.tile as tile
from concourse import bacc, mybir
from concourse.bass_utils import run_bass_kernel_spmd

B, T, E, H, OH = 64, 512, 256, 512, 1024
G = 4 * H          # 2048 gates
BL = 16            # batch per core
M_CH = G // 128    # 16 gate chunks
K_CH = H // 128    # 4 hidden chunks
E_CH = E // 128    # 2 input chunks
MO_CH = OH // 128  # 8
MT_CH = T // 128   # 4
NBLK = 32          # xp block (timesteps); 16 proj units per block

f32 = mybir.dt.float32
f16 = mybir.dt.float16
AF = mybir.ActivationFunctionType
ALU = mybir.AluOpType

# gate permutation: torch order [i,f,g,o] -> kernel order [i,f,o,g]
# perm[new_pos] = old_index  (applied to rows of Wih/Whh and bias)
_GPERM = np.concatenate([
    np.arange(0, H),          # i
    np.arange(H, 2 * H),      # f
    np.arange(3 * H, 4 * H),  # o
    np.arange(2 * H, 3 * H),  # g
])


def _bc_free(ap, reps, width):
    """AP that broadcasts a [P, width] slice to [P, reps, width] via stride-0."""
    return bass.AP(
        tensor=ap.tensor,
        offset=ap.offset,
        ap=[ap.ap[0], [0, reps]] + list(ap.ap[1:]),
    )


def build_nc(t_steps=T, use_collective=True):
    assert t_steps % NBLK == 0
    n_blocks = t_steps // NBLK
    nc = bacc.Bacc("TRN2", target_bir_lowering=False, num_devices=8)

    # ---- DRAM parameters (per-core payloads prepared on host) ----
    xt = nc.declare_dram_parameter("xt", [E_CH, 128, BL * T], f16, isOutput=False)
    wih = nc.declare_dram_parameter("wih", [E_CH, 128, G], f16, isOutput=False)
    whh = nc.declare_dram_parameter("whh", [K_CH, 128, G], f16, isOutput=False)
    biasT = nc.declare_dram_parameter("biasT", [128, M_CH], f32, isOutput=False)
    sc = nc.declare_dram_parameter("sc", [128, T, BL], f16, isOutput=False)
    ident = nc.declare_dram_parameter("ident", [128, 128], f16, isOutput=False)
    w1t = nc.declare_dram_parameter("w1t", [K_CH, 128, OH], f16, isOutput=False)
    b1T = nc.declare_dram_parameter("b1T", [128, MO_CH], f32, isOutput=False)
    w2t = nc.declare_dram_parameter("w2t", [MO_CH, 128, T], f16, isOutput=False)
    b2T = nc.declare_dram_parameter("b2T", [128, MT_CH], f32, isOutput=False)

    out_logits = nc.declare_dram_parameter("out_logits", [128, MT_CH * BL], f32,
                                           isOutput=True)
    out_pooled = nc.declare_dram_parameter("out_pooled", [128, K_CH * BL], f32,
                                           isOutput=True)

    ar_in = nc.dram_tensor("ar_in", [128, MO_CH * BL], f32)
    ar_out = nc.dram_tensor("ar_out", [128, MO_CH * BL], f32)

    with tile.TileContext(nc) as tc:
        with tc.tile_pool(name="const", bufs=1) as const_pool:
            whh_sb = const_pool.tile([128, K_CH, G], f16)
            for k in range(K_CH):
                nc.sync.dma_start(out=whh_sb[:, k, :], in_=whh[k])
            wih_sb = const_pool.tile([128, E_CH, G], f16)
            for k in range(E_CH):
                nc.sync.dma_start(out=wih_sb[:, k, :], in_=wih[k])
            xt_sb = const_pool.tile([128, E_CH, BL * T], f16)
            for k in range(E_CH):
                nc.sync.dma_start(out=xt_sb[:, k, :], in_=xt[k])
            biasT_sb = const_pool.tile([128, M_CH], f32)
            nc.sync.dma_start(out=biasT_sb, in_=biasT[:, :])
            sc_sb = const_pool.tile([128, T, BL], f16)
            nc.sync.dma_start(out=sc_sb, in_=sc[:, :, :])
            ident_sb = const_pool.tile([128, 128], f16)
            nc.sync.dma_start(out=ident_sb, in_=ident[:, :])
            w1t_sb = const_pool.tile([128, K_CH, OH], f16)
            for k in range(K_CH):
                nc.sync.dma_start(out=w1t_sb[:, k, :], in_=w1t[k])
            b1T_sb = const_pool.tile([128, MO_CH], f32)
            nc.sync.dma_start(out=b1T_sb, in_=b1T[:, :])
            w2t_sb = const_pool.tile([128, MO_CH, T], f16)
            for k in range(MO_CH):
                nc.sync.dma_start(out=w2t_sb[:, k, :], in_=w2t[k])
            b2T_sb = const_pool.tile([128, MT_CH], f32)
            nc.sync.dma_start(out=b2T_sb, in_=b2T[:, :])

            # ---- recurrence + streamed projection ----
            # Two sub-batches of 8 sequences (A: b=0..7, B: b=8..15) run the
            # recurrence pipelined half a step out of phase: while A's
            # activation chain runs, B's matmuls stream on the PE.
            SB = BL // 2  # 8
            with tc.tile_pool(name="state", bufs=1) as state_pool:
                h_g = [state_pool.tile([128, K_CH * SB], f16, name=f"h{g}")
                       for g in range(2)]
                c_g = [state_pool.tile([128, K_CH * SB], f16, name=f"c{g}")
                       for g in range(2)]
                acc_g = [state_pool.tile([128, K_CH * SB], f32, name=f"acc{g}")
                         for g in range(2)]
                # double-buffered xp blocks, t-major, group-major cols:
                # col = tt*256 + grp*128 + m*8 + bb
                xpbA = state_pool.tile([128, NBLK, M_CH * BL], f16)
                xpbB = state_pool.tile([128, NBLK, M_CH * BL], f16)
                for g in range(2):
                    nc.vector.memset(h_g[g], 0.0)
                    nc.vector.memset(c_g[g], 0.0)
                    nc.gpsimd.memset(acc_g[g], 0.0)

                with tc.tile_pool(name="rec_ps", bufs=1, space="PSUM") as rec_ps, \
                     tc.tile_pool(name="work", bufs=1) as work:

                    def proj_mms(m, nb, par):
                        """PE half of the xp projection for gate-chunk m of
                        block nb; returns the PSUM tile for the evac."""
                        c0 = nb * BL * NBLK
                        ps = rec_ps.tile([128, NBLK, 2, SB], f32,
                                         tag=f"prj{par}", bufs=1)
                        for k in range(E_CH):
                            nc.tensor.matmul(
                                ps,
                                lhsT=wih_sb[:, k, m * 128:(m + 1) * 128],
                                rhs=xt_sb[:, k, c0:c0 + BL * NBLK],
                                start=(k == 0),
                                stop=(k == E_CH - 1),
                            )
                        return ps

                    def proj_evac(ps, m, xpb_dst):
                        """bias add + f16 cast on DVE, emitted at sub-step B's
                        end so it never blocks the c-chain in the FIFO. Splits
                        unit m's 16 batch cols into the group-major layout."""
                        dst = bass.AP(
                            tensor=xpb_dst.tensor,
                            offset=xpb_dst[:, :, m * SB:(m + 1) * SB].offset,
                            ap=[xpb_dst.ap[0], xpb_dst.ap[1],
                                [128, 2], [1, SB]],
                        )
                        nc.vector.tensor_scalar(
                            out=dst, in0=ps,
                            scalar1=biasT_sb[:, m:m + 1], scalar2=None,
                            op0=ALU.add,
                        )

                    # prologue: project block 0 into xpbA
                    for m in range(M_CH):
                        proj_evac(proj_mms(m, 0, m % 2), m, xpbA)

                    def substep(g, t, xpb, prj_pre=None, prj_post=None):
                        """One recurrence step for sub-batch g (8 seqs)."""
                        h_sb, c_sb = h_g[g], c_g[g]
                        go = g * 128  # xpb col offset of this group
                        pifF = rec_ps.tile([128, 512], f32,
                                           tag=f"pif{g}", bufs=1)
                        psgF = rec_ps.tile([128, 512], f32,
                                           tag=f"psg{g}", bufs=1)
                        pif = pifF[:, 0:96]   # [i,f | o] gates
                        psg = psgF[:, 0:32]
                        tt = t % NBLK
                        # xp folds: no h dependency; run during prev tail
                        nc.tensor.matmul(pif, lhsT=ident_sb,
                                         rhs=xpb[:, tt, go:go + 96],
                                         start=True, stop=False)
                        nc.tensor.matmul(psg, lhsT=ident_sb,
                                         rhs=xpb[:, tt, go + 96:go + 128],
                                         start=True, stop=False)
                        if prj_pre is not None:
                            prj_pre()
                        # i,f group (m=0..7) then o group (m=8..11): one
                        # sigmoid covers all three
                        for j, m in enumerate(range(0, 12)):
                            for k in range(K_CH):
                                nc.tensor.matmul(
                                    pif[:, j * SB:(j + 1) * SB],
                                    lhsT=whh_sb[:, k, m * 128:(m + 1) * 128],
                                    rhs=h_sb[:, k * SB:(k + 1) * SB],
                                    start=False, stop=(k == K_CH - 1),
                                )
                        sif = work.tile([128, 96], f16,
                                        tag=f"sif{g}", bufs=1)
                        nc.scalar.activation(out=sif, in_=pif, func=AF.Sigmoid)
                        t2 = work.tile([128, 32], f16,
                                       tag=f"t2{g}", bufs=1)
                        nc.vector.tensor_mul(out=t2, in0=sif[:, 32:64],
                                             in1=c_sb)
                        # g group (m=12..15)
                        for j, m in enumerate(range(12, 16)):
                            for k in range(K_CH):
                                nc.tensor.matmul(
                                    psg[:, j * SB:(j + 1) * SB],
                                    lhsT=whh_sb[:, k, m * 128:(m + 1) * 128],
                                    rhs=h_sb[:, k * SB:(k + 1) * SB],
                                    start=False, stop=(k == K_CH - 1),
                                )
                        tg = work.tile([128, 32], f16,
                                       tag=f"tg{g}", bufs=1)
                        nc.scalar.activation(out=tg, in_=psg, func=AF.Tanh)
                        t1 = work.tile([128, 32], f16,
                                       tag=f"t1{g}", bufs=1)
                        nc.vector.tensor_mul(out=t1, in0=sif[:, 0:32], in1=tg)
                        nc.vector.tensor_add(out=c_sb, in0=t1, in1=t2)
                        tch = work.tile([128, 32], f16,
                                        tag=f"tch{g}", bufs=1)
                        nc.scalar.activation(out=tch, in_=c_sb, func=AF.Tanh)
                        nc.vector.tensor_mul(out=h_sb, in0=sif[:, 64:96],
                                             in1=tch)
                        if prj_post is not None:
                            prj_post()

                        pt = work.tile([128, 32], f32,
                                       tag=f"pt{g}", bufs=1)
                        nc.gpsimd.tensor_mul(
                            out=pt, in0=h_sb,
                            in1=_bc_free(sc_sb[:, t, g * SB:(g + 1) * SB],
                                         K_CH, SB),
                        )
                        nc.gpsimd.tensor_add(out=acc_g[g], in0=acc_g[g],
                                             in1=pt)

                    for blk in range(n_blocks):
                        xpb = xpbA if blk % 2 == 0 else xpbB
                        xpb_next = xpbB if blk % 2 == 0 else xpbA
                        for tt in range(NBLK):
                            t = blk * NBLK + tt
                            # streamed projection of block blk+1: PE matmuls
                            # ride in sub-batch A's tail slot; the Scalar
                            # evac is emitted inside sub-batch B.
                            prj_a = prj_b = None
                            if tt % 2 == 0 and blk + 1 < n_blocks:
                                u = tt // 2
                                psu = []

                                def prj_a(u=u, psu=psu, blk=blk):
                                    psu.append(proj_mms(u, blk + 1, u % 2))

                                def prj_b(u=u, psu=psu, xpb_next=xpb_next):
                                    proj_evac(psu[0], u, xpb_next)

                            substep(0, t, xpb, prj_pre=prj_a)
                            substep(1, t, xpb, prj_post=prj_b)

                # ---- head ----
                with tc.tile_pool(name="head", bufs=1) as head, \
                     tc.tile_pool(name="head_ps", bufs=1, space="PSUM") as head_ps:
                    # combine sub-batch accumulators: col k*16 + g*8 + bb
                    acc_all = head.tile([128, K_CH, BL], f32)
                    for g in range(2):
                        src = acc_g[g][:, :]
                        src3 = bass.AP(tensor=src.tensor, offset=src.offset,
                                       ap=[src.ap[0], [SB, K_CH], [1, SB]])
                        nc.vector.tensor_copy(
                            out=acc_all[:, :, g * SB:(g + 1) * SB], in_=src3)
                    nc.sync.dma_start(out=out_pooled[:, :], in_=acc_all)
                    acch = head.tile([128, K_CH * BL], f16)
                    nc.vector.tensor_copy(out=acch, in_=acc_all)
                    ps1 = head_ps.tile([128, MO_CH * BL], f32)
                    for mo in range(MO_CH):
                        for k in range(K_CH):
                            nc.tensor.matmul(
                                ps1[:, mo * BL:(mo + 1) * BL],
                                lhsT=w1t_sb[:, k, mo * 128:(mo + 1) * 128],
                                rhs=acch[:, k * BL:(k + 1) * BL],
                                start=(k == 0), stop=(k == K_CH - 1),
                            )
                    p1_sb = head.tile([128, MO_CH * BL], f32)
                    nc.vector.tensor_copy(out=p1_sb, in_=ps1)
                    if use_collective:
                        nc.sync.dma_start(out=ar_in[:, :], in_=p1_sb)
                        nc.gpsimd.collective_compute(
                            "AllReduce",
                            ALU.add,
                            replica_groups=[[0, 1], [2, 3], [4, 5], [6, 7]],
                            ins=[ar_in[:, :].opt()],
                            outs=[ar_out[:, :].opt()],
                        )
                        r_sb = head.tile([128, MO_CH * BL], f32)
                        nc.sync.dma_start(out=r_sb, in_=ar_out[:, :])
                    else:
                        r_sb = p1_sb
                    h1 = head.tile([128, MO_CH * BL], f16)
                    for mo in range(MO_CH):
                        nc.scalar.activation(
                            out=h1[:, mo * BL:(mo + 1) * BL],
                            in_=r_sb[:, mo * BL:(mo + 1) * BL],
                            func=AF.Relu,
                            bias=b1T_sb[:, mo:mo + 1],
                        )
                    ps2 = head_ps.tile([128, MT_CH * BL], f32)
                    for mt in range(MT_CH):
                        for ko in range(MO_CH):
                            nc.tensor.matmul(
                                ps2[:, mt * BL:(mt + 1) * BL],
                                lhsT=w2t_sb[:, ko, mt * 128:(mt + 1) * 128],
                                rhs=h1[:, ko * BL:(ko + 1) * BL],
                                start=(ko == 0), stop=(ko == MO_CH - 1),
                            )
                    lg_sb = head.tile([128, MT_CH * BL], f32)
                    for mt in range(MT_CH):
                        nc.vector.tensor_scalar(
                            out=lg_sb[:, mt * BL:(mt + 1) * BL],
                            in0=ps2[:, mt * BL:(mt + 1) * BL],
                            scalar1=b2T_sb[:, mt:mt + 1], scalar2=None,
                            op0=ALU.add,
                        )
                    nc.sync.dma_start(out=out_logits[:, :], in_=lg_sb)

    nc.compile()
    return nc


def _tile_kxg(w, n_k):
    """[G, K] weight (already permuted rows) -> [n_k, 128, G] fp16 with
    out[k, kk, g] = w[g, k*128+kk]."""
    K = n_k * 128
    wt = w.T.astype(np.float32)  # [K, G]
    return np.ascontiguousarray(
        wt.reshape(n_k, 128, -1)).astype(np.float16)


def prep_core_inputs(x_dir, wih_p, whh_p, bias_p, sc_tb, fc1_w, fc1_b,
                     fc2_w, fc2_b, direction):
    """Build the per-core input map. x_dir [BL, T, E] f32 (already reversed for
    bwd), weights already gate-permuted."""
    ins = {}
    # xt [E_CH, 128, BL*T], cols t-major within NBLK blocks:
    # xt[k][kk][nb*BL*NBLK + tt*BL + b] = x_dir[b, nb*NBLK+tt, k*128+kk]
    nb_tot = T // NBLK
    xtt = x_dir.reshape(BL, nb_tot, NBLK, E_CH, 128)
    xtt = xtt.transpose(3, 4, 1, 2, 0).reshape(E_CH, 128, BL * T)
    ins["xt"] = np.ascontiguousarray(xtt).astype(np.float16)
    ins["wih"] = _tile_kxg(wih_p, E_CH)
    ins["whh"] = _tile_kxg(whh_p, K_CH)
    ins["biasT"] = np.ascontiguousarray(
        bias_p.reshape(M_CH, 128).T).astype(np.float32)
    # sc [128, T, BL] replicated over partitions
    ins["sc"] = np.broadcast_to(
        sc_tb.astype(np.float16)[None, :, :], (128, T, BL)).copy()
    ins["ident"] = np.eye(128, dtype=np.float16)
    w1d = fc1_w[:, direction * H:(direction + 1) * H]  # [OH, H]
    ins["w1t"] = _tile_kxg(w1d, K_CH)
    ins["b1T"] = np.ascontiguousarray(
        fc1_b.reshape(MO_CH, 128).T).astype(np.float32)
    ins["w2t"] = _tile_kxg(fc2_w, MO_CH)
    ins["b2T"] = np.ascontiguousarray(
        fc2_b.reshape(MT_CH, 128).T).astype(np.float32)
    return ins


_NC_CACHE = {}
LAST_RESULT = None


def kernel(x, lengths, attn_w, Wih_f, Whh_f, bih_f, bhh_f,
           Wih_b, Whh_b, bih_b, bhh_b, fc1_w, fc1_b, fc2_w, fc2_b):
    x = np.asarray(x, np.float32)
    lengths = np.asarray(lengths, np.int32)
    attn_w = np.asarray(attn_w, np.float32)
    use_collective = os.environ.get("LSTM_NO_COLLECTIVE", "0") != "1"

    key = (T, use_collective)
    if key not in _NC_CACHE:
        _NC_CACHE[key] = build_nc(T, use_collective)
    nc = _NC_CACHE[key]

    # softmax over attn_w (host glue, exact fp32 as in reference)
    aw = attn_w - attn_w.max()
    e = np.exp(aw)
    scores = (e / e.sum()).astype(np.float32)  # [T]

    tr = np.arange(T)
    # forward sc: sc_f[t, b] = scores[t] * (t < len_b)
    # backward sc: sc_b[tau, b] = scores[len_b-1-tau] * (tau < len_b)
    in_maps = []
    for g in range(4):
        bsl = slice(g * BL, (g + 1) * BL)
        xg = x[bsl]                      # [BL, T, E]
        lg = lengths[bsl]                # [BL]
        mask = tr[:, None] < lg[None, :]  # [T, BL]
        sc_f = scores[:, None] * mask
        idx = np.clip(lg[None, :] - 1 - tr[:, None], 0, T - 1)  # [T, BL]
        sc_b = scores[idx] * mask
        # x reversed per sequence (zeros past length)
        idxc = np.clip(lg[:, None] - 1 - tr[None, :], 0, T - 1)  # [BL, T]
        xrev = np.take_along_axis(xg, idxc[:, :, None], axis=1)
        xrev = xrev * mask.T[:, :, None]

        bias_f = (bih_f + bhh_f)[_GPERM].astype(np.float32)
        bias_b = (bih_b + bhh_b)[_GPERM].astype(np.float32)
        in_maps.append(prep_core_inputs(
            xg, Wih_f[_GPERM], Whh_f[_GPERM], bias_f, sc_f,
            fc1_w, fc1_b, fc2_w, fc2_b, 0))
        in_maps.append(prep_core_inputs(
            xrev, Wih_b[_GPERM], Whh_b[_GPERM], bias_b, sc_b,
            fc1_w, fc1_b, fc2_w, fc2_b, 1))

    trace = os.environ.get("LSTM_TRACE", "0") == "1"
    res = run_bass_kernel_spmd(nc, in_maps, list(range(8)), trace=trace)
    results = res.results
    global LAST_RESULT
    LAST_RESULT = res

    out = np.empty((B, T), np.float32)
    for g in range(4):
        if use_collective:
            lt = results[2 * g]["out_logits"]  # [128, MT_CH*BL]
            lg_out = lt.reshape(128, MT_CH, BL).transpose(2, 1, 0).reshape(BL, T)
        else:
            # host head from pooled partials
            pf = results[2 * g]["out_pooled"]
            pb = results[2 * g + 1]["out_pooled"]
            pooled = np.concatenate(
                [pf.reshape(128, K_CH, BL).transpose(2, 1, 0).reshape(BL, H),
                 pb.reshape(128, K_CH, BL).transpose(2, 1, 0).reshape(BL, H)],
                axis=1)
            h1 = np.maximum(pooled @ fc1_w.T + fc1_b, 0.0)
            lg_out = h1 @ fc2_w.T + fc2_b
        out[g * BL:(g + 1) * BL] = lg_out
    tmask = tr[None, :] < lengths[:, None]
    return np.where(tmask, out, np.float32(-1e30)).astype(np.float32)


# revision 37
# speedup vs baseline: 1.5467x; 1.5467x over previous
"""Trainium2 Bass kernel for bidirectional masked-LSTM + attention pooling + FC head.

Problem (hardcoded shapes): B=64, T=512, E=256, H=512, OH=1024.
  - x [B,T,E] f32, lengths [B] i32, attn_w [T] f32
  - per-direction LSTM weights Wih [4H,E], Whh [4H,H], biases [4H]
  - fc1 [OH,2H]+[OH], fc2 [T,OH]+[T]
  - out: logits [B,T] f32, padded positions = -1e30

Sharding: 8 cores = 4 batch groups (16 seqs) x 2 directions. Each core runs one
direction's full 512-step recurrence for its 16 sequences. Attention pooling is
folded into the recurrence as a masked weighted accumulate (per-(t,b) scale
table precomputed on host, which also implements sequence reversal masking for
the backward direction). The FC head runs on every core; forward/backward
pooled partials are combined with a pairwise AllReduce.

Layouts (per core):
  h "hidden-tiled" [128, K_CH*16]: h[b, hid] at partition hid%128, col (hid//128)*16+b.
  gates PSUM tiled [128, m*16+b] per gate-chunk m (gate g=m*128+p), gate order
  permuted to [i, f, o, g] so i,f share one sigmoid and g is one tanh.

Schedule: the input projection xp = x@Wih.T + bias is streamed INSIDE the
recurrence: while block nb executes, block nb+1's projection matmuls run in
the PE's idle tail slots and the PSUM->SBUF evacuation (bias add, f16 cast)
runs on the idle Vector engine, writing straight into the inactive xpb SBUF
buffer (no DRAM round trip). Per step, the xp addend is folded into PSUM via
an identity matmul (runs during the previous step's elementwise tail), then
gate groups run i,f -> g -> o so sigmoid(i,f), f*c, tanh(g), i*g and the
c-update overlap the remaining matmul groups; only sigmoid(o) -> tanh(c) -> h
stays exposed. Each PSUM tile is padded to a full 2KB bank so the per-parity
tiles live in distinct banks (matmul start=True clears a whole bank).
"""

import os

import numpy as np

import concourse.bass as bass
import concourse.tile as tile
from concourse import bacc, mybir
from concourse.bass_utils import run_bass_kernel_spmd

B, T, E, H, OH = 64, 512, 256, 512, 1024
G = 4 * H          # 2048 gates
BL = 16            # batch per core
M_CH = G // 128    # 16 gate chunks
K_CH = H // 128    # 4 hidden chunks
E_CH = E // 128    # 2 input chunks
MO_CH = OH // 128  # 8
MT_CH = T // 128   # 4
NBLK = 64          # xp block (timesteps); 16 proj units per block

f32 = mybir.dt.float32
f16 = mybir.dt.float16
AF = mybir.ActivationFunctionType
ALU = mybir.AluOpType

# gate permutation: torch order [i,f,g,o] -> kernel order [i,f,o,g]
# perm[new_pos] = old_index  (applied to rows of Wih/Whh and bias)
_GPERM = np.concatenate([
    np.arange(0, H),          # i
    np.arange(H, 2 * H),      # f
    np.arange(3 * H, 4 * H),  # o
    np.arange(2 * H, 3 * H),  # g
])


def _bc_free(ap, reps, width):
    """AP that broadcasts a [P, width] slice to [P, reps, width] via stride-0."""
    return bass.AP(
        tensor=ap.tensor,
        offset=ap.offset,
        ap=[ap.ap[0], [0, reps]] + list(ap.ap[1:]),
    )


def build_nc(t_steps=T, use_collective=True):
    assert t_steps % NBLK == 0
    n_blocks = t_steps // NBLK
    nc = bacc.Bacc("TRN2", target_bir_lowering=False, num_devices=8)

    # ---- DRAM parameters (per-core payloads prepared on host) ----
    xt = nc.declare_dram_parameter("xt", [E_CH, 128, BL * T], f16, isOutput=False)
    wih = nc.declare_dram_parameter("wih", [E_CH, 128, G], f16, isOutput=False)
    whh = nc.declare_dram_parameter("whh", [K_CH, 128, G], f16, isOutput=False)
    biasT = nc.declare_dram_parameter("biasT", [128, M_CH], f32, isOutput=False)
    sc = nc.declare_dram_parameter("sc", [128, T, BL], f16, isOutput=False)
    ident = nc.declare_dram_parameter("ident", [128, 128], f16, isOutput=False)
    w1t = nc.declare_dram_parameter("w1t", [K_CH, 128, OH], f16, isOutput=False)
    b1T = nc.declare_dram_parameter("b1T", [128, MO_CH], f32, isOutput=False)
    w2t = nc.declare_dram_parameter("w2t", [MO_CH, 128, T], f16, isOutput=False)
    b2T = nc.declare_dram_parameter("b2T", [128, MT_CH], f32, isOutput=False)

    out_logits = nc.declare_dram_parameter("out_logits", [128, MT_CH * BL], f32,
                                           isOutput=True)
    out_pooled = nc.declare_dram_parameter("out_pooled", [128, K_CH * BL], f32,
                                           isOutput=True)

    ar_in = nc.dram_tensor("ar_in", [128, MO_CH * BL], f32)
    ar_out = nc.dram_tensor("ar_out", [128, MO_CH * BL], f32)

    with tile.TileContext(nc) as tc:
        with tc.tile_pool(name="const", bufs=1) as const_pool:
            whh_sb = const_pool.tile([128, K_CH, G], f16)
            for k in range(K_CH):
                nc.sync.dma_start(out=whh_sb[:, k, :], in_=whh[k])
            wih_sb = const_pool.tile([128, E_CH, G], f16)
            for k in range(E_CH):
                nc.sync.dma_start(out=wih_sb[:, k, :], in_=wih[k])
            xt_sb = const_pool.tile([128, E_CH, BL * T], f16)
            for k in range(E_CH):
                nc.sync.dma_start(out=xt_sb[:, k, :], in_=xt[k])
            biasT_sb = const_pool.tile([128, M_CH], f32)
            nc.sync.dma_start(out=biasT_sb, in_=biasT[:, :])
            sc_sb = const_pool.tile([128, T, BL], f16)
            nc.sync.dma_start(out=sc_sb, in_=sc[:, :, :])
            ident_sb = const_pool.tile([128, 128], f16)
            nc.sync.dma_start(out=ident_sb, in_=ident[:, :])
            w1t_sb = const_pool.tile([128, K_CH, OH], f16)
            for k in range(K_CH):
                nc.sync.dma_start(out=w1t_sb[:, k, :], in_=w1t[k])
            b1T_sb = const_pool.tile([128, MO_CH], f32)
            nc.sync.dma_start(out=b1T_sb, in_=b1T[:, :])
            w2t_sb = const_pool.tile([128, MO_CH, T], f16)
            for k in range(MO_CH):
                nc.sync.dma_start(out=w2t_sb[:, k, :], in_=w2t[k])
            b2T_sb = const_pool.tile([128, MT_CH], f32)
            nc.sync.dma_start(out=b2T_sb, in_=b2T[:, :])

            # ---- recurrence + streamed projection ----
            with tc.tile_pool(name="state", bufs=1) as state_pool:
                h_sb = state_pool.tile([128, K_CH * BL], f16)
                c_sb = state_pool.tile([128, K_CH * BL], f16)
                acc = state_pool.tile([128, K_CH * BL], f32)
                # double-buffered xp blocks, t-major: col tt*256 + m*16 + b
                xpbA = state_pool.tile([128, NBLK, M_CH * BL], f16)
                xpbB = state_pool.tile([128, NBLK, M_CH * BL], f16)
                nc.vector.memset(h_sb, 0.0)
                nc.vector.memset(c_sb, 0.0)
                nc.gpsimd.memset(acc, 0.0)

                with tc.tile_pool(name="rec_ps", bufs=1, space="PSUM") as rec_ps, \
                     tc.tile_pool(name="work", bufs=1) as work:

                    HB = NBLK // 2  # proj unit covers a half block (1 bank)

                    def proj_mms(u, nb, par):
                        """PE half of the xp projection for unit u (gate-chunk
                        u//2, half-block u%2) of block nb; returns the PSUM
                        tile for the evac."""
                        m, hb = u // 2, u % 2
                        c0 = nb * BL * NBLK + hb * BL * HB
                        ps = rec_ps.tile([128, HB, BL], f32, tag=f"prj{par}",
                                         bufs=1)
                        for k in range(E_CH):
                            nc.tensor.matmul(
                                ps,
                                lhsT=wih_sb[:, k, m * 128:(m + 1) * 128],
                                rhs=xt_sb[:, k, c0:c0 + BL * HB],
                                start=(k == 0),
                                stop=(k == E_CH - 1),
                            )
                        return ps

                    def proj_evac(ps, u, xpb_dst):
                        """bias add + f16 cast on the Scalar engine (keeps the
                        Vector FIFO free for the c-chain)."""
                        m, hb = u // 2, u % 2
                        nc.scalar.activation(
                            out=xpb_dst[:, hb * HB:(hb + 1) * HB,
                                        m * BL:(m + 1) * BL], in_=ps,
                            func=AF.Identity,
                            bias=biasT_sb[:, m:m + 1], scale=1.0,
                        )

                    # prologue: project block 0 into xpbA
                    for u in range(2 * M_CH):
                        proj_evac(proj_mms(u, 0, u % 2), u, xpbA)

                    for blk in range(n_blocks):
                        xpb = xpbA if blk % 2 == 0 else xpbB
                        xpb_next = xpbB if blk % 2 == 0 else xpbA
                        for tt in range(NBLK):
                            t = blk * NBLK + tt
                            par = t % 2
                            # full-bank PSUM tiles (start=True clears a bank)
                            pifF = rec_ps.tile([128, 512], f32,
                                               tag=f"pif{par}", bufs=1)
                            psgF = rec_ps.tile([128, 512], f32,
                                               tag=f"psg{par}", bufs=1)
                            psoF = rec_ps.tile([128, 512], f32,
                                               tag=f"pso{par}", bufs=1)
                            pif, psg, pso = (pifF[:, 0:128], psgF[:, 0:64],
                                             psoF[:, 0:64])
                            # xp folds: no h dependency; run during prev tail
                            nc.tensor.matmul(pif, lhsT=ident_sb,
                                             rhs=xpb[:, tt, 0:128],
                                             start=True, stop=False)
                            nc.tensor.matmul(psg, lhsT=ident_sb,
                                             rhs=xpb[:, tt, 192:256],
                                             start=True, stop=False)
                            nc.tensor.matmul(pso, lhsT=ident_sb,
                                             rhs=xpb[:, tt, 128:192],
                                             start=True, stop=False)
                            # i,f group (m=0..7)
                            for m in range(8):
                                for k in range(K_CH):
                                    nc.tensor.matmul(
                                        pif[:, m * 16:(m + 1) * 16],
                                        lhsT=whh_sb[:, k, m * 128:(m + 1) * 128],
                                        rhs=h_sb[:, k * BL:(k + 1) * BL],
                                        start=False, stop=(k == K_CH - 1),
                                    )
                            sif = work.tile([128, 128], f16,
                                            tag=f"sif{par}", bufs=1)
                            nc.scalar.activation(out=sif, in_=pif, func=AF.Sigmoid)
                            t2 = work.tile([128, 64], f16,
                                           tag=f"t2{par}", bufs=1)
                            nc.vector.tensor_mul(out=t2, in0=sif[:, 64:128],
                                                 in1=c_sb)
                            # g group (m=12..15)
                            for j, m in enumerate(range(12, 16)):
                                for k in range(K_CH):
                                    nc.tensor.matmul(
                                        psg[:, j * 16:(j + 1) * 16],
                                        lhsT=whh_sb[:, k, m * 128:(m + 1) * 128],
                                        rhs=h_sb[:, k * BL:(k + 1) * BL],
                                        start=False, stop=(k == K_CH - 1),
                                    )
                            tg = work.tile([128, 64], f16,
                                           tag=f"tg{par}", bufs=1)
                            nc.scalar.activation(out=tg, in_=psg, func=AF.Tanh)
                            t1 = work.tile([128, 64], f16,
                                           tag=f"t1{par}", bufs=1)
                            nc.vector.tensor_mul(out=t1, in0=sif[:, 0:64], in1=tg)
                            nc.vector.tensor_add(out=c_sb, in0=t1, in1=t2)
                            # o group (m=8..11)
                            for j, m in enumerate(range(8, 12)):
                                for k in range(K_CH):
                                    nc.tensor.matmul(
                                        pso[:, j * 16:(j + 1) * 16],
                                        lhsT=whh_sb[:, k, m * 128:(m + 1) * 128],
                                        rhs=h_sb[:, k * BL:(k + 1) * BL],
                                        start=False, stop=(k == K_CH - 1),
                                    )
                            # streamed projection of block blk+1 (1 unit per
                            # 2 steps) — emitted after the o group so the PE
                            # reaches it only in the tail idle window
                            prj_ps = None
                            if tt % 2 == 0 and blk + 1 < n_blocks:
                                prj_ps = proj_mms(tt // 2, blk + 1,
                                                  (tt // 2) % 2)
                            so = work.tile([128, 64], f16,
                                           tag=f"so{par}", bufs=1)
                            nc.scalar.activation(out=so, in_=pso, func=AF.Sigmoid)
                            tch = work.tile([128, 64], f16,
                                            tag=f"tch{par}", bufs=1)
                            nc.scalar.activation(out=tch, in_=c_sb, func=AF.Tanh)
                            nc.vector.tensor_mul(out=h_sb, in0=so, in1=tch)
                            if prj_ps is not None:
                                proj_evac(prj_ps, tt // 2, xpb_next)

                            pt = work.tile([128, 64], f32,
                                           tag=f"pt{par}", bufs=1)
                            nc.gpsimd.tensor_mul(
                                out=pt, in0=h_sb,
                                in1=_bc_free(sc_sb[:, t, :], K_CH, BL),
                            )
                            nc.gpsimd.tensor_add(out=acc, in0=acc, in1=pt)

                # ---- head ----
                with tc.tile_pool(name="head", bufs=1) as head, \
                     tc.tile_pool(name="head_ps", bufs=1, space="PSUM") as head_ps:
                    nc.sync.dma_start(out=out_pooled[:, :], in_=acc)
                    acch = head.tile([128, K_CH * BL], f16)
                    nc.vector.tensor_copy(out=acch, in_=acc)
                    ps1 = head_ps.tile([128, MO_CH * BL], f32)
                    for mo in range(MO_CH):
                        for k in range(K_CH):
                            nc.tensor.matmul(
                                ps1[:, mo * BL:(mo + 1) * BL],
                                lhsT=w1t_sb[:, k, mo * 128:(mo + 1) * 128],
                                rhs=acch[:, k * BL:(k + 1) * BL],
                                start=(k == 0), stop=(k == K_CH - 1),
                            )
                    p1_sb = head.tile([128, MO_CH * BL], f32)
                    nc.vector.tensor_copy(out=p1_sb, in_=ps1)
                    if use_collective:
                        nc.sync.dma_start(out=ar_in[:, :], in_=p1_sb)
                        nc.gpsimd.collective_compute(
                            "AllReduce",
                            ALU.add,
                            replica_groups=[[0, 1], [2, 3], [4, 5], [6, 7]],
                            ins=[ar_in[:, :].opt()],
                            outs=[ar_out[:, :].opt()],
                        )
                        r_sb = head.tile([128, MO_CH * BL], f32)
                        nc.sync.dma_start(out=r_sb, in_=ar_out[:, :])
                    else:
                        r_sb = p1_sb
                    h1 = head.tile([128, MO_CH * BL], f16)
                    for mo in range(MO_CH):
                        nc.scalar.activation(
                            out=h1[:, mo * BL:(mo + 1) * BL],
                            in_=r_sb[:, mo * BL:(mo + 1) * BL],
                            func=AF.Relu,
                            bias=b1T_sb[:, mo:mo + 1],
                        )
                    ps2 = head_ps.tile([128, MT_CH * BL], f32)
                    for mt in range(MT_CH):
                        for ko in range(MO_CH):
                            nc.tensor.matmul(
                                ps2[:, mt * BL:(mt + 1) * BL],
                                lhsT=w2t_sb[:, ko, mt * 128:(mt + 1) * 128],
                                rhs=h1[:, ko * BL:(ko + 1) * BL],
                                start=(ko == 0), stop=(ko == MO_CH - 1),
                            )
                    lg_sb = head.tile([128, MT_CH * BL], f32)
                    for mt in range(MT_CH):
                        nc.vector.tensor_scalar(
                            out=lg_sb[:, mt * BL:(mt + 1) * BL],
                            in0=ps2[:, mt * BL:(mt + 1) * BL],
                            scalar1=b2T_sb[:, mt:mt + 1], scalar2=None,
                            op0=ALU.add,
                        )
                    nc.sync.dma_start(out=out_logits[:, :], in_=lg_sb)

    nc.compile()
    return nc


def _tile_kxg(w, n_k):
    """[G, K] weight (already permuted rows) -> [n_k, 128, G] fp16 with
    out[k, kk, g] = w[g, k*128+kk]."""
    K = n_k * 128
    wt = w.T.astype(np.float32)  # [K, G]
    return np.ascontiguousarray(
        wt.reshape(n_k, 128, -1)).astype(np.float16)


def prep_core_inputs(x_dir, wih_p, whh_p, bias_p, sc_tb, fc1_w, fc1_b,
                     fc2_w, fc2_b, direction):
    """Build the per-core input map. x_dir [BL, T, E] f32 (already reversed for
    bwd), weights already gate-permuted."""
    ins = {}
    # xt [E_CH, 128, BL*T], cols t-major within NBLK blocks:
    # xt[k][kk][nb*BL*NBLK + tt*BL + b] = x_dir[b, nb*NBLK+tt, k*128+kk]
    nb_tot = T // NBLK
    xtt = x_dir.reshape(BL, nb_tot, NBLK, E_CH, 128)
    xtt = xtt.transpose(3, 4, 1, 2, 0).reshape(E_CH, 128, BL * T)
    ins["xt"] = np.ascontiguousarray(xtt).astype(np.float16)
    ins["wih"] = _tile_kxg(wih_p, E_CH)
    ins["whh"] = _tile_kxg(whh_p, K_CH)
    ins["biasT"] = np.ascontiguousarray(
        bias_p.reshape(M_CH, 128).T).astype(np.float32)
    # sc [128, T, BL] replicated over partitions
    ins["sc"] = np.broadcast_to(
        sc_tb.astype(np.float16)[None, :, :], (128, T, BL)).copy()
    ins["ident"] = np.eye(128, dtype=np.float16)
    w1d = fc1_w[:, direction * H:(direction + 1) * H]  # [OH, H]
    ins["w1t"] = _tile_kxg(w1d, K_CH)
    ins["b1T"] = np.ascontiguousarray(
        fc1_b.reshape(MO_CH, 128).T).astype(np.float32)
    ins["w2t"] = _tile_kxg(fc2_w, MO_CH)
    ins["b2T"] = np.ascontiguousarray(
        fc2_b.reshape(MT_CH, 128).T).astype(np.float32)
    return ins


_NC_CACHE = {}
LAST_RESULT = None


def kernel(x, lengths, attn_w, Wih_f, Whh_f, bih_f, bhh_f,
           Wih_b, Whh_b, bih_b, bhh_b, fc1_w, fc1_b, fc2_w, fc2_b):
    x = np.asarray(x, np.float32)
    lengths = np.asarray(lengths, np.int32)
    attn_w = np.asarray(attn_w, np.float32)
    use_collective = os.environ.get("LSTM_NO_COLLECTIVE", "0") != "1"

    key = (T, use_collective)
    if key not in _NC_CACHE:
        _NC_CACHE[key] = build_nc(T, use_collective)
    nc = _NC_CACHE[key]

    # softmax over attn_w (host glue, exact fp32 as in reference)
    aw = attn_w - attn_w.max()
    e = np.exp(aw)
    scores = (e / e.sum()).astype(np.float32)  # [T]

    tr = np.arange(T)
    # forward sc: sc_f[t, b] = scores[t] * (t < len_b)
    # backward sc: sc_b[tau, b] = scores[len_b-1-tau] * (tau < len_b)
    in_maps = []
    for g in range(4):
        bsl = slice(g * BL, (g + 1) * BL)
        xg = x[bsl]                      # [BL, T, E]
        lg = lengths[bsl]                # [BL]
        mask = tr[:, None] < lg[None, :]  # [T, BL]
        sc_f = scores[:, None] * mask
        idx = np.clip(lg[None, :] - 1 - tr[:, None], 0, T - 1)  # [T, BL]
        sc_b = scores[idx] * mask
        # x reversed per sequence (zeros past length)
        idxc = np.clip(lg[:, None] - 1 - tr[None, :], 0, T - 1)  # [BL, T]
        xrev = np.take_along_axis(xg, idxc[:, :, None], axis=1)
        xrev = xrev * mask.T[:, :, None]

        bias_f = (bih_f + bhh_f)[_GPERM].astype(np.float32)
        bias_b = (bih_b + bhh_b)[_GPERM].astype(np.float32)
        in_maps.append(prep_core_inputs(
            xg, Wih_f[_GPERM], Whh_f[_GPERM], bias_f, sc_f,
            fc1_w, fc1_b, fc2_w, fc2_b, 0))
        in_maps.append(prep_core_inputs(
            xrev, Wih_b[_GPERM], Whh_b[_GPERM], bias_b, sc_b,
            fc1_w, fc1_b, fc2_w, fc2_b, 1))

    trace = os.environ.get("LSTM_TRACE", "0") == "1"
    res = run_bass_kernel_spmd(nc, in_maps, list(range(8)), trace=trace)
    results = res.results
    global LAST_RESULT
    LAST_RESULT = res

    out = np.empty((B, T), np.float32)
    for g in range(4):
        if use_collective:
            lt = results[2 * g]["out_logits"]  # [128, MT_CH*BL]
            lg_out = lt.reshape(128, MT_CH, BL).transpose(2, 1, 0).reshape(BL, T)
        else:
            # host head from pooled partials
            pf = results[2 * g]["out_pooled"]
            pb = results[2 * g + 1]["out_pooled"]
            pooled = np.concatenate(
                [pf.reshape(128, K_CH, BL).transpose(2, 1, 0).reshape(BL, H),
                 pb.reshape(128, K_CH, BL).transpose(2, 1, 0).reshape(BL, H)],
                axis=1)
            h1 = np.maximum(pooled @ fc1_w.T + fc1_b, 0.0)
            lg_out = h1 @ fc2_w.T + fc2_b
        out[g * BL:(g + 1) * BL] = lg_out
    tmask = tr[None, :] < lengths[:, None]
    return np.where(tmask, out, np.float32(-1e30)).astype(np.float32)
